# revision 1
# baseline (speedup 1.0000x reference)
"""Trainium2 Bass kernel for nn_CustomLinear (learned-twiddle butterfly net).

Math (verified vs reference in fp32, rel err ~4e-7):
  reference pads x [2048,4096] to [2048,8192], half-swaps (XOR N/2), then 13
  radix-2 butterfly stages with learned twiddles tw_s[i] = exp(-2j*pi/N *
  k*w[k]), k = i*N/step.  After the half-swap the first 4096 elements of each
  row are ZERO, so:
    - stages 1..7  == one 128x128 complex matrix M applied to each of the 32
      nonzero 128-blocks (input real -> 2 real matmuls)
    - stages 8..12 == butterflies on the 4096-element nonzero subvector
    - stage 13     == out = [t, -t],  t = tw13 * v   (lo half is zero)

Device layout: transposed blocks [e=partition, (j,r)=free] so twiddles are
per-partition scalars (scalar_tensor_tensor / ACT scale).  PE does the block
transposes in/out and the M matmuls; DVE/ACT/GPSIMD share butterflies+copies.

Sharding: pure data parallel, batch 2048 -> 8 cores x 256 rows.
"""
import numpy as np
from contextlib import ExitStack

import concourse.bacc as bacc
import concourse.mybir as mybir
from concourse.tile import TileContext
from concourse.bass_utils import run_bass_kernel_spmd

N = 8192
B = 2048
IN_F = 4096
NCORES = 8
B_CORE = B // NCORES          # 256 rows per core
NTILES = B_CORE // 128        # 2 row-tiles of 128 rows
NBLK = 32                     # nonzero 128-blocks per row
F32 = mybir.dt.float32

# const tensor column layout
_MRE, _MIM, _ID = 0, 128, 256
_TWR, _TWI = 384, 415         # 31 cols each (stages 8..12)
_T13C, _T13S = 446, 478       # 32 cols each
CW_W = 512

_CACHE = {}


def _stage_tw(s, w):
    step = 1 << s
    half = step >> 1
    k = np.arange(half) * (N // step)
    ang = (-2.0 * np.pi / N) * k.astype(np.float64) * w[k].astype(np.float64)
    return np.exp(1j * ang)


def _host_consts(w):
    M = np.eye(128, dtype=np.complex128)
    for s in range(1, 8):
        step = 1 << s
        half = step >> 1
        tw = _stage_tw(s, w)
        Bm = np.zeros((step, step), np.complex128)
        Bm[:half, :half] = np.eye(half)
        Bm[:half, half:] = np.diag(tw)
        Bm[half:, :half] = np.eye(half)
        Bm[half:, half:] = -np.diag(tw)
        M = np.kron(np.eye(128 // step), Bm) @ M

    cw = np.zeros((128, CW_W), np.float32)
    cw[:, _MRE:_MRE + 128] = M.real.T.astype(np.float32)
    cw[:, _MIM:_MIM + 128] = M.imag.T.astype(np.float32)
    cw[:, _ID:_ID + 128] = np.eye(128, dtype=np.float32)
    off = 0
    for s in range(8, 13):
        tw = _stage_tw(s, w)
        hb = 1 << (s - 8)           # hi width in blocks
        for jr in range(hb):
            cw[:, _TWR + off] = tw.real[jr * 128:(jr + 1) * 128]
            cw[:, _TWI + off] = tw.imag[jr * 128:(jr + 1) * 128]
            off += 1
    tw13 = _stage_tw(13, w)
    for j in range(NBLK):
        cw[:, _T13C + j] = tw13.real[j * 128:(j + 1) * 128]
        cw[:, _T13S + j] = tw13.imag[j * 128:(j + 1) * 128]
    return cw


def _build_program():
    nc = bacc.Bacc("TRN2", target_bir_lowering=False, debug=False)
    x_d = nc.dram_tensor("x", [B_CORE, IN_F], F32, kind="ExternalInput").ap()
    cw_d = nc.dram_tensor("cw", [128, CW_W], F32, kind="ExternalInput").ap()
    y_d = nc.dram_tensor("y", [B_CORE, 2 * N], F32, kind="ExternalOutput").ap()

    AL = mybir.AluOpType

    with TileContext(nc) as tc, ExitStack() as ctx:
        cpool = ctx.enter_context(tc.tile_pool(name="const", bufs=1))
        xpool = ctx.enter_context(tc.tile_pool(name="xin", bufs=2))
        xtpool = ctx.enter_context(tc.tile_pool(name="xt", bufs=2))
        tpool = ctx.enter_context(tc.tile_pool(name="t13", bufs=1))
        zpool = ctx.enter_context(tc.tile_pool(name="z", bufs=1))
        spool = ctx.enter_context(tc.tile_pool(name="scr", bufs=2))
        opool = ctx.enter_context(tc.tile_pool(name="out", bufs=4))
        ps_t = ctx.enter_context(tc.tile_pool(name="ps_t", bufs=2, space="PSUM"))
        ps_m = ctx.enter_context(tc.tile_pool(name="ps_m", bufs=2, space="PSUM"))
        ps_o = ctx.enter_context(tc.tile_pool(name="ps_o", bufs=2, space="PSUM"))

        cw = cpool.tile([128, CW_W], F32)
        nc.sync.dma_start(cw[:], cw_d[:])
        ident = cw[:, _ID:_ID + 128]
        mre_t = cw[:, _MRE:_MRE + 128]
        mim_t = cw[:, _MIM:_MIM + 128]

        for ti in range(NTILES):
            r0 = ti * 128
            # ---- load + transpose 32 blocks:  XT[e, j*128+r] ----
            xin = xpool.tile([128, IN_F], F32, tag="xin")
            nc.sync.dma_start(xin[:, :2048], x_d[r0:r0 + 128, :2048])
            nc.sync.dma_start(xin[:, 2048:], x_d[r0:r0 + 128, 2048:])
            xt = xtpool.tile([128, IN_F], F32, tag="xt")
            for q in range(NBLK // 4):
                pt = ps_t.tile([128, 512], F32, tag="pt")
                for k in range(4):
                    j = q * 4 + k
                    nc.tensor.transpose(pt[:, k * 128:(k + 1) * 128],
                                        xin[:, j * 128:(j + 1) * 128], ident)
                nc.scalar.copy(xt[:, q * 512:(q + 1) * 512], pt[:])

            # ---- phase A matmuls: Z = M @ blocks ----
            zre = zpool.tile([128, IN_F], F32, tag="zre")
            zim = zpool.tile([128, IN_F], F32, tag="zim")
            for c in range(8):
                sl = slice(c * 512, (c + 1) * 512)
                pm = ps_m.tile([128, 512], F32, tag="pm")
                nc.tensor.matmul(pm[:], mre_t, xt[:, sl], start=True, stop=True)
                nc.scalar.copy(zre[:, sl], pm[:])
                pm2 = ps_m.tile([128, 512], F32, tag="pm")
                nc.tensor.matmul(pm2[:], mim_t, xt[:, sl], start=True, stop=True)
                nc.scalar.copy(zim[:, sl], pm2[:])

            # ---- butterfly stages 8..12 (in place on zre/zim) ----
            col = 0
            for s in range(8, 13):
                G = 1 << (s - 7)          # group width in blocks
                hb = G // 2
                ng = NBLK // G
                z3r = zre[:].rearrange("p (g c) -> p g c", g=ng)
                z3i = zim[:].rearrange("p (g c) -> p g c", g=ng)
                for jr in range(hb):
                    twr = cw[:, _TWR + col:_TWR + col + 1]
                    twi = cw[:, _TWI + col:_TWI + col + 1]
                    col += 1
                    lo = slice(jr * 128, (jr + 1) * 128)
                    hi = slice((hb + jr) * 128, (hb + jr + 1) * 128)
                    for g0 in range(0, ng, 8):
                        g1 = min(g0 + 8, ng)
                        ngc = g1 - g0
                        L = ngc * 128
                        lo_r, hi_r = z3r[:, g0:g1, lo], z3r[:, g0:g1, hi]
                        lo_i, hi_i = z3i[:, g0:g1, lo], z3i[:, g0:g1, hi]
                        tm1 = spool.tile([128, L], F32, tag="tm1")
                        tm2 = spool.tile([128, L], F32, tag="tm2")
                        tre = spool.tile([128, L], F32, tag="tre")
                        tim = spool.tile([128, L], F32, tag="tim")
                        v1 = tm1[:].rearrange("p (g c) -> p g c", g=ngc)
                        v2 = tm2[:].rearrange("p (g c) -> p g c", g=ngc)
                        vr = tre[:].rearrange("p (g c) -> p g c", g=ngc)
                        vi = tim[:].rearrange("p (g c) -> p g c", g=ngc)
                        nc.scalar.mul(v1, hi_i, twi)
                        nc.scalar.mul(v2, hi_i, twr)
                        nc.vector.scalar_tensor_tensor(
                            vr, hi_r, twr, v1, op0=AL.mult, op1=AL.subtract)
                        nc.vector.scalar_tensor_tensor(
                            vi, hi_r, twi, v2, op0=AL.mult, op1=AL.add)
                        # hi' = lo - t first (reads lo), then lo' += t
                        nc.gpsimd.tensor_tensor(hi_r, lo_r, vr, op=AL.subtract)
                        nc.gpsimd.tensor_tensor(hi_i, lo_i, vi, op=AL.subtract)
                        nc.vector.tensor_tensor(lo_r, lo_r, vr, op=AL.add)
                        nc.vector.tensor_tensor(lo_i, lo_i, vi, op=AL.add)

            # ---- stage 13: t = tw13 * v  (per block j) ----
            tr13 = tpool.tile([128, IN_F], F32, tag="tr13")
            ti13 = tpool.tile([128, IN_F], F32, tag="ti13")
            for j in range(NBLK):
                bs = slice(j * 128, (j + 1) * 128)
                ccol = cw[:, _T13C + j:_T13C + j + 1]
                scol = cw[:, _T13S + j:_T13S + j + 1]
                tm1 = spool.tile([128, 128], F32, tag="tm1")
                tm2 = spool.tile([128, 128], F32, tag="tm2")
                nc.scalar.mul(tm1[:], zim[:, bs], scol)
                nc.scalar.mul(tm2[:], zim[:, bs], ccol)
                nc.vector.scalar_tensor_tensor(
                    tr13[:, bs], zre[:, bs], ccol, tm1[:],
                    op0=AL.mult, op1=AL.subtract)
                nc.vector.scalar_tensor_tensor(
                    ti13[:, bs], zre[:, bs], scol, tm2[:],
                    op0=AL.mult, op1=AL.add)

            # ---- transpose back + interleave re/im, write [t, -t] ----
            for jc in range(NBLK // 4):       # chunks of 4 blocks
                op_ = opool.tile([128, 1024], F32, tag="op")
                on_ = opool.tile([128, 1024], F32, tag="on")
                pr = ps_o.tile([128, 512], F32, tag="por")
                pi = ps_o.tile([128, 512], F32, tag="poi")
                for k in range(4):
                    j = jc * 4 + k
                    bs = slice(j * 128, (j + 1) * 128)
                    nc.tensor.transpose(pr[:, k * 128:(k + 1) * 128],
                                        tr13[:, bs], ident)
                    nc.tensor.transpose(pi[:, k * 128:(k + 1) * 128],
                                        ti13[:, bs], ident)
                vp = op_[:].rearrange("p (f two) -> p f two", two=2)
                vn = on_[:].rearrange("p (f two) -> p f two", two=2)
                if jc % 2 == 0:
                    nc.vector.tensor_copy(vp[:, :, 0], pr[:])
                    nc.scalar.copy(vp[:, :, 1], pi[:])
                    nc.vector.tensor_scalar_mul(vn[:, :, 0], pr[:], -1.0)
                    nc.scalar.mul(vn[:, :, 1], pi[:], -1.0)
                else:
                    nc.scalar.copy(vp[:, :, 0], pr[:])
                    nc.vector.tensor_copy(vp[:, :, 1], pi[:])
                    nc.scalar.mul(vn[:, :, 0], pr[:], -1.0)
                    nc.vector.tensor_scalar_mul(vn[:, :, 1], pi[:], -1.0)
                c0 = jc * 1024
                nc.sync.dma_start(y_d[r0:r0 + 128, c0:c0 + 1024], op_[:])
                nc.sync.dma_start(
                    y_d[r0:r0 + 128, N + c0:N + c0 + 1024], on_[:])

    nc.compile()
    return nc


def kernel(x: np.ndarray, weights: np.ndarray) -> np.ndarray:
    x = np.ascontiguousarray(np.asarray(x, dtype=np.float32))
    w = np.asarray(weights, dtype=np.float32)
    if "nc" not in _CACHE:
        _CACHE["nc"] = _build_program()
    nc = _CACHE["nc"]
    cw = _host_consts(w)
    in_maps = [
        {"x": x[ci * B_CORE:(ci + 1) * B_CORE], "cw": cw}
        for ci in range(NCORES)
    ]
    res = run_bass_kernel_spmd(nc, in_maps, list(range(NCORES)))
    _CACHE["last_results"] = res
    out = np.concatenate([res.results[ci]["y"] for ci in range(NCORES)], axis=0)
    return out.view(np.complex64)



# revision 14
# speedup vs baseline: 2.1269x; 2.1269x over previous
"""Trainium2 Bass kernel for nn_CustomLinear (learned-twiddle butterfly net).

Math (validated vs reference in numpy, rel err ~5e-16):
  reference pads x [2048,4096] to [2048,8192], half-swaps (XOR N/2), then 13
  radix-2 butterfly stages with learned twiddles.  After the half-swap the lo
  half is zero, so the nonzero 4096-vector goes through:
    - stages 1..7  == one 128x128 complex matrix M per 128-block
    - stage  8     == adds only; its twiddle is FOLDED into M for odd
                     blocks (M_od = diag(tw8) @ M)
    - stage  9     == elementwise butterflies (per-partition scalar twiddles)
    - stages 10..12== twiddle product on PE as diagonal matmuls, adds on DVE
    - stage 13     == out = [t, -t], t = c13 * v: folded into the
                     transpose-out matmuls (rhs = per-block diag(c13)); only
                     t is written; the host materializes [t, -t].

Everything on device is bf16 (matmuls accumulate fp32 in PSUM). Correctness
gate is max-normalized rel_err < 2e-2; bf16 lands ~1e-3.

Sharding: pure data parallel, batch 2048 -> 8 cores x 256 rows.
"""
import numpy as np
import ml_dtypes
from contextlib import ExitStack

import concourse.bacc as bacc
import concourse.mybir as mybir
from concourse.tile import TileContext
from concourse.bass_utils import run_bass_kernel_spmd

N = 8192
B = 2048
IN_F = 4096
NCORES = 8
B_CORE = B // NCORES          # 256 rows per core
NTILES = B_CORE // 128        # 2 row-tiles of 128 rows
NBLK = 32                     # nonzero 128-blocks per row
BF = mybir.dt.bfloat16
F32 = mybir.dt.float32
NPBF = ml_dtypes.bfloat16

PE_STAGES = (9, 10, 11, 12)   # stages whose twiddle mult runs on PE

# ---- cwa column layout (ident + M + stage-9..12 twiddle columns) ----
_ID = 0
_MEVR, _MEVI, _MODR, _MODI = 128, 256, 384, 512
_TWR, _TWI = 640, 670         # 30 cols each (stages 9..12, jr-major)
CWA_W = 704

# ---- cwd: diag tiles [Dre | Dim | -Dim] per jr-set; PE stages then st13 ----
_NSET_S = sum(1 << (s - 8) for s in PE_STAGES)
_D13 = _NSET_S * 384
CWD_W = _D13 + NBLK * 384

_CACHE = {}


def _stage_tw(s, w):
    step = 1 << s
    half = step >> 1
    k = np.arange(half) * (N // step)
    ang = (-2.0 * np.pi / N) * k.astype(np.float64) * w[k].astype(np.float64)
    return np.exp(1j * ang)


def _host_consts(w):
    M = np.eye(128, dtype=np.complex128)
    for s in range(1, 8):
        step = 1 << s
        half = step >> 1
        tw = _stage_tw(s, w)
        Bm = np.zeros((step, step), np.complex128)
        Bm[:half, :half] = np.eye(half)
        Bm[:half, half:] = np.diag(tw)
        Bm[half:, :half] = np.eye(half)
        Bm[half:, half:] = -np.diag(tw)
        M = np.kron(np.eye(128 // step), Bm) @ M
    tw8 = _stage_tw(8, w)
    M_od = np.diag(tw8) @ M

    cwa = np.zeros((128, CWA_W), np.float32)
    cwa[:, _ID:_ID + 128] = np.eye(128, dtype=np.float32)
    # lhsT tiles: lhsT[e, e'] = M[e', e]  (out = lhsT.T @ rhs = M @ rhs)
    cwa[:, _MEVR:_MEVR + 128] = M.real.T
    cwa[:, _MEVI:_MEVI + 128] = M.imag.T
    cwa[:, _MODR:_MODR + 128] = M_od.real.T
    cwa[:, _MODI:_MODI + 128] = M_od.imag.T
    off = 0
    for s in range(9, 13):
        tw = _stage_tw(s, w)
        hb = 1 << (s - 8)
        for jr in range(hb):
            cwa[:, _TWR + off] = tw.real[jr * 128:(jr + 1) * 128]
            cwa[:, _TWI + off] = tw.imag[jr * 128:(jr + 1) * 128]
            off += 1

    cwd = np.zeros((128, CWD_W), np.float32)
    ii = np.arange(128)
    off = 0
    for s in PE_STAGES:
        tw = _stage_tw(s, w)
        for jr in range(1 << (s - 8)):
            dre = tw.real[jr * 128:(jr + 1) * 128]
            dim = tw.imag[jr * 128:(jr + 1) * 128]
            cwd[ii, off + ii] = dre
            cwd[ii, off + 128 + ii] = dim
            cwd[ii, off + 256 + ii] = -dim
            off += 384
    c13 = _stage_tw(13, w)
    for j in range(NBLK):
        dre = c13.real[j * 128:(j + 1) * 128]
        dim = c13.imag[j * 128:(j + 1) * 128]
        cwd[ii, off + ii] = dre
        cwd[ii, off + 128 + ii] = dim
        cwd[ii, off + 256 + ii] = -dim
        off += 384
    return cwa.astype(NPBF), cwd.astype(NPBF)


def _dset_off(s):
    off = 0
    for t in PE_STAGES:
        if t == s:
            return off
        off += (1 << (t - 8)) * 384
    raise ValueError(s)


def _build_program():
    nc = bacc.Bacc("TRN2", target_bir_lowering=False, debug=False)
    x_d = nc.dram_tensor("x", [B_CORE, IN_F], BF, kind="ExternalInput").ap()
    cwa_d = nc.dram_tensor("cwa", [128, CWA_W], BF, kind="ExternalInput").ap()
    cwd_d = nc.dram_tensor("cwd", [128, CWD_W], BF, kind="ExternalInput").ap()
    y_d = nc.dram_tensor("y", [B_CORE, 2 * IN_F], BF, kind="ExternalOutput").ap()

    AL = mybir.AluOpType

    with TileContext(nc) as tc, ExitStack() as ctx:
        cpool = ctx.enter_context(tc.tile_pool(name="const", bufs=1))
        xpool = ctx.enter_context(tc.tile_pool(name="xin", bufs=1))
        xtpool = ctx.enter_context(tc.tile_pool(name="xt", bufs=1))
        zpool = ctx.enter_context(tc.tile_pool(name="z", bufs=2))
        tpool = ctx.enter_context(tc.tile_pool(name="t", bufs=1))
        opool = ctx.enter_context(tc.tile_pool(name="out", bufs=4))
        ps = ctx.enter_context(tc.tile_pool(name="ps", bufs=4, space="PSUM"))

        cwa = cpool.tile([128, CWA_W], BF)
        nc.sync.dma_start(cwa[:], cwa_d[:])
        cwd = cpool.tile([128, CWD_W], BF)
        xins = []
        for ti in range(NTILES):
            xin = xpool.tile([128, IN_F], BF, tag=f"xin{ti}")
            nc.sync.dma_start(xin[:, :2048],
                              x_d[ti * 128:ti * 128 + 128, :2048])
            nc.sync.dma_start(xin[:, 2048:],
                              x_d[ti * 128:ti * 128 + 128, 2048:])
            xins.append(xin)
        nc.sync.dma_start(cwd[:, :_D13], cwd_d[:, :_D13])
        nc.sync.dma_start(cwd[:, _D13:], cwd_d[:, _D13:])
        ident = cwa[:, _ID:_ID + 128]
        mevr = cwa[:, _MEVR:_MEVR + 128]
        mevi = cwa[:, _MEVI:_MEVI + 128]
        modr = cwa[:, _MODR:_MODR + 128]
        modi = cwa[:, _MODI:_MODI + 128]

        def dset(s, j):
            o = (_D13 if s == 13 else _dset_off(s)) + j * 384
            return (cwd[:, o:o + 128],
                    cwd[:, o + 128:o + 256],
                    cwd[:, o + 256:o + 384])

        for ti in range(NTILES):
            r0 = ti * 128
            # ---- transpose 32 blocks: xt[e, (j,r)] (bf16 via bitcast) ----
            xin = xins[ti]
            xt = xtpool.tile([128, IN_F], BF, tag=f"xt{ti}")
            for q in range(4):                       # rounds of 8 blocks
                pq = ps.tile([128, 512], F32, tag=f"pm{ti}")
                pt = pq[:].bitcast(BF)               # [128, 1024] bf16 view
                for k in range(8):
                    j = q * 8 + k
                    nc.tensor.transpose(pt[:, k * 128:(k + 1) * 128],
                                        xin[:, j * 128:(j + 1) * 128], ident)
                nc.vector.tensor_copy(xt[:, q * 1024:(q + 1) * 1024], pt[:])

            # ---- phase A matmuls (stages 1..8), strided 4-pair chunks ----
            zre = zpool.tile([128, IN_F], BF, tag="zre")
            zim = zpool.tile([128, IN_F], BF, tag="zim")
            xv = xt[:].rearrange("p (g c) -> p g c", g=16)
            zvr = zre[:].rearrange("p (g c) -> p g c", g=16)
            zvi = zim[:].rearrange("p (g c) -> p g c", g=16)
            t8r = tpool.tile([128, 2048], BF, tag=f"t8r{ti}")
            t8i = tpool.tile([128, 2048], BF, tag=f"t8i{ti}")
            t8rv = t8r[:].rearrange("p (g c) -> p g c", g=16)
            t8iv = t8i[:].rearrange("p (g c) -> p g c", g=16)
            for c in range(4):                       # chunks of 4 pairs
                g0, g1 = c * 4, c * 4 + 4
                ev = xv[:, g0:g1, 0:128]
                od = xv[:, g0:g1, 128:256]
                p_evr = ps.tile([128, 512], F32, tag=f"pm{ti}")
                p_evi = ps.tile([128, 512], F32, tag=f"pm{ti}")
                p_odr = ps.tile([128, 512], F32, tag=f"pm{ti}")
                p_odi = ps.tile([128, 512], F32, tag=f"pm{ti}")
                nc.tensor.matmul(p_evr[:], mevr, ev, start=True, stop=True)
                nc.tensor.matmul(p_evi[:], mevi, ev, start=True, stop=True)
                nc.tensor.matmul(p_odr[:], modr, od, start=True, stop=True)
                nc.tensor.matmul(p_odi[:], modi, od, start=True, stop=True)
                nc.scalar.copy(zvr[:, g0:g1, 0:128], p_evr[:])
                nc.scalar.copy(zvi[:, g0:g1, 0:128], p_evi[:])
                nc.vector.tensor_copy(t8rv[:, g0:g1, :], p_odr[:])
                nc.scalar.copy(t8iv[:, g0:g1, :], p_odi[:])

            # ---- stage 8 adds (twiddle folded into M_od) ----
            nc.vector.tensor_tensor(zvr[:, :, 128:256], zvr[:, :, 0:128],
                                    t8rv[:, :, :], op=AL.subtract)
            nc.gpsimd.tensor_tensor(zvi[:, :, 128:256], zvi[:, :, 0:128],
                                    t8iv[:, :, :], op=AL.subtract)
            nc.vector.tensor_tensor(zvr[:, :, 0:128], zvr[:, :, 0:128],
                                    t8rv[:, :, :], op=AL.add)
            nc.gpsimd.tensor_tensor(zvi[:, :, 0:128], zvi[:, :, 0:128],
                                    t8iv[:, :, :], op=AL.add)

            # ---- stages 9..12: PE diag twiddles + DVE adds ----
            for s in range(9, 13):
                G = 1 << (s - 7)
                hb = G // 2
                ng = NBLK // G
                z4r = zre[:].rearrange("p (g j e) -> p g j e", g=ng, j=G)
                z4i = zim[:].rearrange("p (g j e) -> p g j e", g=ng, j=G)
                tr = tpool.tile([128, 2048], BF, tag=f"str{ti}")
                ti_ = tpool.tile([128, 2048], BF, tag=f"sti{ti}")
                # t tiles jr-major: [jr, g, e]
                t3r = tr[:].rearrange("p (j g e) -> p g j e", j=hb, g=ng)
                t3i = ti_[:].rearrange("p (j g e) -> p g j e", j=hb, g=ng)

                # 4 units per stage, each: 2 psum tiles + mms + 2 evacs of 512
                w_ = ng * 128                       # per-jr width
                for u in range(4):
                    p_tr = ps.tile([128, 512], F32, tag=f"pm{ti}")
                    p_ti = ps.tile([128, 512], F32, tag=f"pm{ti}")
                    if s == 9:                      # unit = (jr, g-half)
                        jr, h = u // 2, u % 2
                        dre, dim, mdim = dset(s, jr)
                        hr = z4r[:, h * 4:h * 4 + 4, hb + jr, :]
                        hi = z4i[:, h * 4:h * 4 + 4, hb + jr, :]
                        nc.tensor.matmul(p_tr[:], dre, hr, start=True, stop=False)
                        nc.tensor.matmul(p_tr[:], mdim, hi, start=False, stop=True)
                        nc.tensor.matmul(p_ti[:], dim, hr, start=True, stop=False)
                        nc.tensor.matmul(p_ti[:], dre, hi, start=False, stop=True)
                        o0 = jr * 1024 + h * 512
                    else:                           # unit = 512/w_ jr values
                        per = 512 // w_
                        for k in range(per):
                            jr = u * per + k
                            dre, dim, mdim = dset(s, jr)
                            hr = z4r[:, :, hb + jr, :]
                            hi = z4i[:, :, hb + jr, :]
                            sr = p_tr[:, k * w_:(k + 1) * w_]
                            si = p_ti[:, k * w_:(k + 1) * w_]
                            nc.tensor.matmul(sr, dre, hr, start=True, stop=False)
                            nc.tensor.matmul(sr, mdim, hi, start=False, stop=True)
                            nc.tensor.matmul(si, dim, hr, start=True, stop=False)
                            nc.tensor.matmul(si, dre, hi, start=False, stop=True)
                        o0 = u * 512
                    if u % 2 == 0:
                        nc.scalar.copy(tr[:, o0:o0 + 512], p_tr[:])
                        nc.vector.tensor_copy(ti_[:, o0:o0 + 512], p_ti[:])
                    else:
                        nc.vector.tensor_copy(tr[:, o0:o0 + 512], p_tr[:])
                        nc.scalar.copy(ti_[:, o0:o0 + 512], p_ti[:])

                # batched adds on DVE: hi' = lo - t first, then lo' += t
                lo_r, lo_i = z4r[:, :, 0:hb, :], z4i[:, :, 0:hb, :]
                hi_r, hi_i = z4r[:, :, hb:G, :], z4i[:, :, hb:G, :]
                nc.vector.tensor_tensor(hi_r, lo_r, t3r, op=AL.subtract)
                nc.gpsimd.tensor_tensor(hi_i, lo_i, t3i, op=AL.subtract)
                nc.vector.tensor_tensor(lo_r, lo_r, t3r, op=AL.add)
                nc.vector.tensor_tensor(lo_i, lo_i, t3i, op=AL.add)

            # ---- transpose-out with stage-13 fold, interleave, DMA ----
            for jc in range(NBLK // 4):
                p_r = ps.tile([128, 512], F32, tag=f"pm{ti}")
                p_i = ps.tile([128, 512], F32, tag=f"pm{ti}")
                for k in range(4):
                    j = jc * 4 + k
                    bs = slice(j * 128, (j + 1) * 128)
                    dre, dim, mdim = dset(13, j)
                    pr = p_r[:, k * 128:(k + 1) * 128]
                    pi = p_i[:, k * 128:(k + 1) * 128]
                    nc.tensor.matmul(pr, zre[:, bs], dre, start=True, stop=False)
                    nc.tensor.matmul(pr, zim[:, bs], mdim, start=False, stop=True)
                    nc.tensor.matmul(pi, zre[:, bs], dim, start=True, stop=False)
                    nc.tensor.matmul(pi, zim[:, bs], dre, start=False, stop=True)
                op_ = opool.tile([128, 1024], BF, tag="op")
                vp = op_[:].rearrange("p (f two) -> p f two", two=2)
                if jc % 2 == 0:
                    nc.scalar.copy(vp[:, :, 0], p_r[:])
                    nc.vector.tensor_copy(vp[:, :, 1], p_i[:])
                else:
                    nc.vector.tensor_copy(vp[:, :, 0], p_r[:])
                    nc.scalar.copy(vp[:, :, 1], p_i[:])
                c0 = jc * 1024
                nc.sync.dma_start(y_d[r0:r0 + 128, c0:c0 + 1024], op_[:])

    nc.compile()
    return nc


def kernel(x: np.ndarray, weights: np.ndarray) -> np.ndarray:
    x = np.asarray(x, dtype=np.float32)
    w = np.asarray(weights, dtype=np.float32)
    xb = np.ascontiguousarray(x.astype(NPBF))
    if "nc" not in _CACHE:
        _CACHE["nc"] = _build_program()
    nc = _CACHE["nc"]
    cwa, cwd = _host_consts(w)
    in_maps = [
        {"x": xb[ci * B_CORE:(ci + 1) * B_CORE], "cwa": cwa, "cwd": cwd}
        for ci in range(NCORES)
    ]
    res = run_bass_kernel_spmd(nc, in_maps, list(range(NCORES)))
    _CACHE["last_results"] = res
    t = np.concatenate([res.results[ci]["y"] for ci in range(NCORES)], axis=0)
    t = t.astype(np.float32).view(np.complex64)      # [2048, 4096] complex
    return np.concatenate([t, -t], axis=1)           # [2048, 8192]
